# revision 1
# baseline (speedup 1.0000x reference)
"""Trainium2 Bass kernel for nn_ChartEncoder (CKY-style chart encoder).

Strategy (pure data parallelism over batch, 16 batch elements per core):
- The whole chart lives in SBUF, transposed: per level-block tiles
  [H=100 partitions, 16 batch, L cells], dtype float32r (e8m11, full-rate
  PE matmuls, producers round).
- Per level, per split i the left/right children are CONTIGUOUS ranges of
  chart blocks i and level-1-i, so the "gather" is just strided matmul
  reads.  Two accumulating fp32r matmuls with an augmented weight
  [W1half | 0] compute h1 (rows 0..99) and a zero row 100; two more with
  [Ws_half replicated 101x] compute the softmax logit replicated on all
  101 rows.
- ACT applies relu(h1 + b1) (bias row 100 = 1.0 -> ones row) and
  exp(s) (bias shift dropped: softmax is shift-invariant, so `bs` is
  mathematically irrelevant).  GPSIMD multiplies, DVE does the grouped
  reduce (softmax groups are runs of N consecutive pair-columns - the
  reference's idx-major/(L,N)-view quirk).  Row 100 of the product
  reduces to the softmax denominator.
- W2 is applied AFTER the weighted sum (linearity), normalization by
  1/denom after W2 (scalar per group), b2 added last (softmax weights
  sum to 1).
- Output path: PE-transposes chart blocks back to row-major + DMA out.
"""

import threading

import numpy as np

import concourse.bacc as bacc
import concourse.tile as tile
from concourse import mybir
from concourse import bass_utils
from concourse.masks import make_identity

F32 = mybir.dt.float32
F32R = mybir.dt.float32r
AF = mybir.ActivationFunctionType
ALU = mybir.AluOpType
AX = mybir.AxisListType

H = 100
HP = H + 1  # h rows + ones/score row
LEN = 64
BPC = 16  # batch elements per core
NCORES = 8
CHART = LEN * (LEN + 1) // 2  # 2080

_OFF = [CHART - (LEN - l) * (LEN - l + 1) // 2 for l in range(LEN)]

_lock = threading.Lock()
_cache = {}


def _sub_chunks(level, f0, f1):
    """Chunks (i, j0, j1) covering f in [f0, f1), cut at split boundaries
    and into near-equal pieces of <= 32 positions (16*32 = 512 psum cols,
    and >= 256 cols where the split allows - fp32r full-rate threshold)."""
    L = LEN - level
    out = []
    f = f0
    while f < f1:
        i, j0 = divmod(f, L)
        run = min(L - j0, f1 - f)
        npc = -(-run // 32)
        base, rem = divmod(run, npc)
        jj = j0
        for p in range(npc):
            w = base + (1 if p < rem else 0)
            out.append((i, jj, jj + w))
            jj += w
        f += run
    return out


def _build():
    nc = bacc.Bacc(
        "TRN2", target_bir_lowering=False, debug=False, num_devices=NCORES
    )
    x_d = nc.dram_tensor("x", [BPC, LEN, H], F32, kind="ExternalInput")
    w1_d = nc.dram_tensor("w1", [2 * H, H], F32, kind="ExternalInput")
    b1_d = nc.dram_tensor("b1", [H, 1], F32, kind="ExternalInput")
    w2_d = nc.dram_tensor("w2", [H, H], F32, kind="ExternalInput")
    b2_d = nc.dram_tensor("b2", [H, 1], F32, kind="ExternalInput")
    ws_d = nc.dram_tensor("ws", [2 * H, 1], F32, kind="ExternalInput")
    out_d = nc.dram_tensor("out", [BPC, CHART, H], F32, kind="ExternalOutput")

    with tile.TileContext(nc) as tc:
        with (
            tc.tile_pool(name="wts", bufs=1) as wts,
            tc.tile_pool(name="chart", bufs=1) as chart,
            tc.tile_pool(name="lvl", bufs=2) as lvl,
            tc.tile_pool(name="stg", bufs=2) as stg,
            tc.tile_pool(name="ps", bufs=2, space="PSUM") as ps,
            tc.tile_pool(name="ps1", bufs=1, space="PSUM") as ps1,
        ):
            # ---------------- weight prep ----------------
            w1t = wts.tile([H, HP], F32R, tag="w1t")
            w1b = wts.tile([H, HP], F32R, tag="w1b")
            wst = wts.tile([H, HP], F32R, tag="wst")
            wsb = wts.tile([H, HP], F32R, tag="wsb")
            w2s = wts.tile([H, H], F32R, tag="w2s")
            ones1 = wts.tile([1, H], F32R, tag="ones1")
            b1aug = wts.tile([HP, 1], F32, tag="b1aug")
            b2s = wts.tile([H, 1], F32, tag="b2s")
            ident = wts.tile([128, 128], F32, tag="ident")

            make_identity(nc, ident)
            onesstg = wts.tile([1, H], F32, tag="onesstg")
            nc.vector.memset(onesstg, 1.0)
            nc.vector.tensor_copy(out=ones1, in_=onesstg)

            w1stg = wts.tile([H, HP], F32, tag="w1stg")
            w1stg2 = wts.tile([H, HP], F32, tag="w1stg2")
            nc.vector.memset(w1stg, 0.0)
            nc.vector.memset(w1stg2, 0.0)
            nc.sync.dma_start(out=w1stg[:, 0:H], in_=w1_d.ap()[0:H, :])
            nc.sync.dma_start(out=w1stg2[:, 0:H], in_=w1_d.ap()[H:2 * H, :])
            nc.vector.tensor_copy(out=w1t, in_=w1stg)
            nc.vector.tensor_copy(out=w1b, in_=w1stg2)

            wsstg = wts.tile([H, 1], F32, tag="wsstg")
            wsstg2 = wts.tile([H, 1], F32, tag="wsstg2")
            nc.sync.dma_start(out=wsstg, in_=ws_d.ap()[0:H, :])
            nc.sync.dma_start(out=wsstg2, in_=ws_d.ap()[H:2 * H, :])
            nc.vector.tensor_copy(out=wst, in_=wsstg.to_broadcast([H, HP]))
            nc.vector.tensor_copy(out=wsb, in_=wsstg2.to_broadcast([H, HP]))

            w2stg = wts.tile([H, H], F32, tag="w2stg")
            nc.sync.dma_start(out=w2stg, in_=w2_d.ap())
            nc.vector.tensor_copy(out=w2s, in_=w2stg)

            nc.vector.memset(b1aug, 1.0)
            nc.sync.dma_start(out=b1aug[0:H, :], in_=b1_d.ap())
            nc.sync.dma_start(out=b2s, in_=b2_d.ap())

            # ---------------- chart blocks ----------------
            blk = [
                chart.tile([H, BPC, LEN - l], F32R, tag=f"blk{l}", name=f"blk{l}", bufs=1)
                for l in range(LEN)
            ]

            # ---------------- level 0: load x (transposed) ----------------
            # direct DRAM->DRAM copy of x into the output's first 64 cells
            nc.sync.dma_start(out=out_d.ap()[:, 0:LEN, :], in_=x_d.ap())
            x_rows = x_d.ap().rearrange("b p h -> (b p) h")  # [1024, 100]
            for c in range(BPC * LEN // 128):
                xstg = stg.tile([128, H], F32, tag="xstg")
                nc.sync.dma_start(out=xstg, in_=x_rows[c * 128:(c + 1) * 128, :])
                tps = ps.tile([128, 128], F32, tag="tp")
                nc.tensor.transpose(tps[0:H, 0:128], xstg, ident)
                # 128 rows = 2 batch elements' worth of 64 cells
                nc.vector.tensor_copy(
                    out=blk[0][:, 2 * c:2 * c + 2, :], in_=tps[0:H, 0:128]
                )

            # ---------------- out-path emitter ----------------
            def emit_out(l):
                L = LEN - l
                kb = min(BPC, 128 // L)
                b0 = 0
                while b0 < BPC:
                    nb = min(kb, BPC - b0)
                    cells = nb * L
                    src = blk[l][:, b0:b0 + nb, :].bitcast(F32)
                    src = src.rearrange("p b l -> p (b l)")
                    tpo = ps.tile([128, 128], F32, tag="tp")
                    nc.tensor.transpose(tpo[0:cells, 0:H], src, ident[0:H, 0:H])
                    ostg = stg.tile([128, H], F32, tag="ostg", bufs=3)
                    nc.vector.tensor_copy(
                        out=ostg[0:cells, :], in_=tpo[0:cells, 0:H]
                    )
                    nc.sync.dma_start(
                        out=out_d.ap()[b0:b0 + nb, _OFF[l]:_OFF[l] + L, :],
                        in_=ostg[0:cells, :],
                    )
                    b0 += nb

            # ---------------- main level loop ----------------
            for level in range(1, LEN):
                N, L = level, LEN - level
                NL = N * L
                F = min(NL, max(1, 176 // N) * N)
                n_sub = (NL + F - 1) // F

                red = lvl.tile([HP, BPC, L], F32R, tag="red")

                for s in range(n_sub):
                    f0, f1 = s * F, min((s + 1) * F, NL)
                    fw = f1 - f0
                    h1r = lvl.tile([HP, BPC, fw], F32, tag="h1r")
                    ex = lvl.tile([HP, BPC, fw], F32, tag="ex")
                    sub_chunks = _sub_chunks(level, f0, f1)
                    assert sum(c[2] - c[1] for c in sub_chunks) == fw
                    # chunks touching the freshest block (level-1: i==0
                    # r-side, i==N-1 l-side) go last for cross-level overlap
                    sub_chunks.sort(
                        key=lambda c: (c[0] == 0 or c[0] == N - 1, c[0], c[1])
                    )
                    for (i, j0, j1) in sub_chunks:
                        jr = j1 - j0
                        # fp32r matmul ISA restriction: innermost src/dst
                        # element counts must be even -> put the batch dim
                        # (16) innermost in rhs and psum
                        lsl = blk[i][:, :, j0:j1].rearrange("p b j -> p j b")
                        rsl = blk[level - 1 - i][
                            :, :, i + 1 + j0:i + 1 + j1
                        ].rearrange("p b j -> p j b")
                        hps = ps.tile([HP, 32, BPC], F32, tag="hps")
                        sps = ps.tile([HP, 32, BPC], F32, tag="sps")
                        hv = hps[:, 0:jr, :]
                        sv = sps[:, 0:jr, :]
                        nc.tensor.matmul(hv, w1t, lsl, start=True, stop=False)
                        nc.tensor.matmul(hv, w1b, rsl, start=False, stop=True)
                        nc.tensor.matmul(sv, wst, lsl, start=True, stop=False)
                        nc.tensor.matmul(sv, wsb, rsl, start=False, stop=True)
                        fo = i * L + j0 - f0
                        nc.scalar.activation(
                            out=h1r[:, :, fo:fo + jr].rearrange(
                                "p b j -> p j b"
                            ),
                            in_=hv, func=AF.Relu, bias=b1aug, scale=1.0,
                        )
                        nc.scalar.activation(
                            out=ex[:, :, fo:fo + jr].rearrange("p b j -> p j b"),
                            in_=sv, func=AF.Exp,
                        )
                    # weighted products (in-place over ex) and grouped sums
                    nc.vector.tensor_tensor(
                        out=ex, in0=h1r, in1=ex, op=ALU.mult
                    )
                    g0, g1 = f0 // N, f1 // N
                    with nc.allow_low_precision("fp32r rounding for matmul input"):
                        nc.vector.tensor_reduce(
                            out=red[:, :, g0:g1],
                            in_=ex.rearrange("p b (g n) -> p b g n", n=N),
                            axis=AX.X, op=ALU.add,
                        )

                # recip of denominators (row 100).  Compute engines cannot
                # address base partition 100, so bounce the row via DMA.
                denom0 = lvl.tile([1, BPC, L], F32, tag="denom0", bufs=1)
                nc.sync.dma_start(out=denom0, in_=red[H:HP, :, :].bitcast(F32))
                recip = lvl.tile([1, BPC, L], F32R, tag="recip", bufs=1)
                with nc.allow_low_precision("fp32r rounding for matmul input"):
                    nc.vector.reciprocal(out=recip, in_=denom0)

                # W2 + normalize + b2 -> chart block
                red_flat = red[0:H].rearrange("p b l -> p (b l)")
                recip_flat = recip.rearrange("p b l -> p (b l)")
                blk_flat = blk[level].rearrange("p b l -> p (b l)")
                cols = BPC * L
                c0 = 0
                while c0 < cols:
                    cw = min(504, cols - c0)
                    # broadcast 1/denom to 100 partitions via K=1 matmul
                    rps = ps1.tile([H, 504], F32, tag="rps")
                    nc.tensor.matmul(
                        rps[:, 0:cw], ones1, recip_flat[:, c0:c0 + cw],
                        start=True, stop=True,
                    )
                    # normalize the group sums in place (scaling a column
                    # commutes with the W2 contraction over features)
                    with nc.allow_low_precision("fp32r rounding for matmul input"):
                        nc.vector.tensor_tensor(
                            out=red_flat[:, c0:c0 + cw],
                            in0=red_flat[:, c0:c0 + cw], in1=rps[:, 0:cw],
                            op=ALU.mult,
                        )
                    w2ps = ps1.tile([H, 504], F32, tag="w2ps")
                    nc.tensor.matmul(
                        w2ps[:, 0:cw], w2s, red_flat[:, c0:c0 + cw],
                        start=True, stop=True,
                    )
                    nc.scalar.activation(
                        out=blk_flat[:, c0:c0 + cw], in_=w2ps[:, 0:cw],
                        func=AF.Identity, bias=b2s, scale=1.0,
                    )
                    c0 += cw

                emit_out(level)

    nc.compile()
    return nc


def _get_nc():
    with _lock:
        if "nc" not in _cache:
            _cache["nc"] = _build()
        return _cache["nc"]


def _make_in_maps(inputs):
    x = np.ascontiguousarray(inputs["x"], dtype=np.float32)  # [128, 64, 100]
    W1 = np.ascontiguousarray(inputs["W1"], dtype=np.float32)
    b1 = np.ascontiguousarray(inputs["b1"], dtype=np.float32).reshape(H, 1)
    W2 = np.ascontiguousarray(inputs["W2"], dtype=np.float32)
    b2 = np.ascontiguousarray(inputs["b2"], dtype=np.float32).reshape(H, 1)
    Ws = np.ascontiguousarray(inputs["Ws"], dtype=np.float32)
    # bs shifts every softmax logit equally -> cancels; intentionally unused.
    return [
        dict(
            x=np.ascontiguousarray(x[c * BPC:(c + 1) * BPC]),
            w1=W1, b1=b1, w2=W2, b2=b2, ws=Ws,
        )
        for c in range(NCORES)
    ]


def kernel(**inputs):
    nc = _get_nc()
    in_maps = _make_in_maps(inputs)
    res = bass_utils.run_bass_kernel_spmd(
        nc, in_maps, core_ids=list(range(NCORES))
    )
    out = np.concatenate([res.results[c]["out"] for c in range(NCORES)], axis=0)
    return out.astype(np.float32)


if __name__ == "__main__":
    rng = np.random.default_rng(0)
    ins = dict(
        x=rng.standard_normal((128, LEN, H)).astype(np.float32),
        W1=(rng.standard_normal((2 * H, H)) * 0.07).astype(np.float32),
        b1=np.zeros(H, np.float32),
        W2=(rng.standard_normal((H, H)) * 0.1).astype(np.float32),
        b2=np.zeros(H, np.float32),
        Ws=(rng.standard_normal((2 * H, 1)) * 0.07).astype(np.float32),
        bs=np.zeros(1, np.float32),
    )
    out = kernel(**ins)
    print("kernel ran, out shape", out.shape, "finite:", np.isfinite(out).all())

